# revision 2
# baseline (speedup 1.0000x reference)
"""Trainium2 Bass kernel for nn_Nibbler_70755291234540 (gnn_message_passing).

q = concat(obs, relu(per-gvf tiny nets(gathered obs))) @ q_W.T

Strategy (8 NeuronCores, SPMD single program):
  - Shard the 4096 GVFs across cores (512/core); every core sees the
    full batch, produces a partial Q over its gvf features; host sums
    cores and adds the (tiny, 1% of FLOPs) obs-part of the Q head in
    exact f32.
  - 512 gvfs/core = 32 supergroups (SG) of 16 gvfs.  Gathered operand
    per SG: 256 rows = (gvf-in-SG, input-slot), host-pre-gathered into
    fp8 and laid out [128 partitions, 2 k-subtiles, 2048 batch] for
    fp8 DoubleRow matmuls (256-row contraction; dual-fp8 is only legal
    on the full 128x128 PE tile).  One 4KB-descriptor DMA per SG keeps
    the DMA bus saturated (~the bottleneck: ~18.4MB/core).
  - Stage 1 per SG: 8 DoubleRow matmuls (256-col chunks, the dual-fp8
    moving-free limit) into 4 rotating 1-bank PSUM tiles; relu-evicted
    (ACT/DVE alternating) to fp8 feats (= 16*feat_true; W prescaled by
    16 to stay out of the fp8 subnormal range, relu is pos-homogeneous).
  - Q head: feats of 2 SGs form a DoubleRow moving tile [128, 2, 2048];
    stationary 16*q_W slice zero-padded 18->128 cols (full-tile rule).
    Accumulates 256*q into 2 qacc PSUM tiles (batch halves); evicted
    with scale 1/256 to f16 rows 0..17 only.
  - A ~4us dummy-matmul warmup ramps the PE p-state during the DMA
    ramp; the final pair's Q matmuls interleave with its evictions so
    each output half DMAs out as soon as it closes.
"""

import sys
import types

import numpy as np
import ml_dtypes

# ---- problem constants (hardcoded; kernel.py must be self-contained) ----
B = 2048
OBS_DIM = 4096
N_GVFS = 4096
IPG = 16  # inputs per gvf
HPG = 8  # hidden per gvf
NA = 18  # actions
N_CORES = 8
GPC = N_GVFS // N_CORES  # 512 gvfs per core
SGV = 16  # gvfs per supergroup
NSG = GPC // SGV  # 32 supergroups per core
NPAIR = NSG // 2  # 16 SG pairs (Q-head moving tiles)
CH = 256  # matmul output chunk (DoubleRow: moving free <= 512 -> out 256;
# larger compiles but yields garbage on HW)
OWN_OBS = OBS_DIM // N_CORES  # 512 obs dims per core -> 4 f16 obs blocks
OWN_BLKS = OWN_OBS // 128
SW = 16.0  # weight prescale (wbd = SW*W); relu is pos-homogeneous
SQ = 256.0  # accumulated q scale (SW * SW); evicted with 1/SQ
# chunk sizes in SGs: small first chunks so the pipeline starts early,
# small last chunk so the drain chain starts early
CHUNK_SGS = [1, 1, 2, 2, 3, 3, 4, 4, 4, 4, 3, 1]
assert sum(CHUNK_SGS) == NSG


def _install_axon_profile_hook():
    """bass_utils trace=True under axon needs antenv.axon_hooks; shim it."""
    try:
        import antenv
    except ImportError:
        return
    if "antenv.axon_hooks" in sys.modules:
        return
    hooks = types.ModuleType("antenv.axon_hooks")
    hooks._hook = None

    def set_axon_ntff_profile_hook(h):
        hooks._hook = h

    def get_axon_ntff_profile_hook():
        return hooks._hook

    hooks.set_axon_ntff_profile_hook = set_axon_ntff_profile_hook
    hooks.get_axon_ntff_profile_hook = get_axon_ntff_profile_hook
    sys.modules["antenv.axon_hooks"] = hooks
    antenv.axon_hooks = hooks
    try:
        from trn_agent_boot.trn_boot import _ntff_profile_via_ctypes

        hook = _ntff_profile_via_ctypes("/opt/axon/libaxon_pjrt.so")
        if hook is not None:
            set_axon_ntff_profile_hook(hook)
    except Exception:
        pass


_install_axon_profile_hook()

import concourse.bacc as bacc
import concourse.mybir as mybir
import concourse.tile as tile
from concourse.bass_utils import run_bass_kernel_spmd

F16 = mybir.dt.float16
F32 = mybir.dt.float32
F8 = mybir.dt.float8e4
DR = mybir.MatmulPerfMode.DoubleRow

_PROGRAM = None


def _build_program():
    nc = bacc.Bacc(None, target_bir_lowering=False, debug=False, num_devices=N_CORES)

    gath = nc.dram_tensor("gath", [128, NSG, 2, B], F8, kind="ExternalInput")
    wbd = nc.dram_tensor("wbd", [128, NSG, 2, 128], F8, kind="ExternalInput")
    # Q stationary zero-padded NA->128: dual-fp8 (DoubleRow) LDWEIGHTS/MM
    # are only legal on the full 128x128 PE tile (tile_size (128,32) fails
    # walrus 's3_lw_dual_fp8_restrictions'), so the Q head uses the full
    # array with dead columns instead of 32-col strips.
    qwt = nc.dram_tensor("qwt", [128, NPAIR, 2, 128], F8, kind="ExternalInput")
    qp = nc.dram_tensor("qp", [NA, B], F16, kind="ExternalOutput")

    RELU = mybir.ActivationFunctionType.Relu
    COPY = mybir.ActivationFunctionType.Copy

    with tile.TileContext(nc) as tc:
        with (
            tc.tile_pool(name="const", bufs=1) as const,
            tc.tile_pool(name="gbuf", bufs=12) as gbuf,
            tc.tile_pool(name="fbuf", bufs=4) as fbuf,
            tc.tile_pool(name="qout", bufs=1) as qout,
            tc.tile_pool(name="pre_ps", bufs=4, space="PSUM") as pre_ps,
            tc.tile_pool(name="qacc_ps", bufs=1, space="PSUM") as qacc_ps,
        ):
            wbd_sb = const.tile([128, NSG, 2, 128], F8)
            qwt_sb = const.tile([128, NPAIR, 2, 128], F8)
            # SG 0-1 stationaries first so chunk-0 compute isn't stuck
            # behind the full weight transfer
            nc.sync.dma_start(wbd_sb[:, 0:2, :, :], wbd[:, 0:2, :, :])

            qaccs = [
                qacc_ps.tile([128, B // 2], F32, tag=f"qacc{h}", name=f"qacc{h}")
                for h in range(2)
            ]

            evict_ctr = [0]

            def evict(dst, src):
                """relu + fp8-quantize a [128, 512] PSUM tile (ACT/DVE
                alternating; GPSIMD can't read PSUM on TRN2)."""
                if evict_ctr[0] % 2 == 0:
                    nc.scalar.activation(dst, src, RELU)
                else:
                    nc.vector.tensor_scalar_max(dst, src, 0.0)
                evict_ctr[0] += 1

            def emit_qout(h):
                # split ACT/DVE so the final eviction takes ~0.45us not 0.9
                qsb = qout.tile([NA, B // 2], F16, name=f"qsb{h}")
                nc.scalar.activation(
                    qsb[:, 0:512], qaccs[h][0:NA, 0:512], COPY, scale=1.0 / SQ
                )
                nc.vector.tensor_scalar_mul(
                    qsb[:, 512:1024], qaccs[h][0:NA, 512:1024], 1.0 / SQ
                )
                nc.sync.dma_start(qp[:, h * (B // 2) : (h + 1) * (B // 2)], qsb[:])

            def q_mm(t, qmov, k):
                h, kk = k // 4, k % 4
                nc.tensor.matmul(
                    qaccs[h][:, CH * kk : CH * (kk + 1)],
                    qwt_sb[:, t, :, :],
                    qmov[:, :, CH * k : CH * (k + 1)],
                    start=(t == 0 and kk % 2 == 0),
                    stop=(t == NPAIR - 1 and kk % 2 == 1),
                    perf_mode=DR,
                    tile_position=(0, 0),
                    skip_group_check=True,
                )

            def q_flush(t, qmov):
                """Q-head DoubleRow matmuls for SG pair t (full-array)."""
                for k in range(8):
                    q_mm(t, qmov, k)

            # PE p-state warm-up: ~4us of dummy matmuls during the DMA ramp
            # so the PE clock is at full speed when the real stream starts
            # (measured: removing this costs ~5us of slow early matmuls).
            warm = const.tile([128, 256], F16, name="warm")
            nc.vector.memset(warm[:], 0.0)
            wpre = pre_ps.tile([128, 512], F32, tag="pre", name="wpre")
            for w in range(48):
                nc.tensor.matmul(
                    wpre[:, 0:256],
                    warm[:, 0:128],
                    warm[:],
                    start=True,
                    stop=True,
                    tile_position=(0, 0),
                    skip_group_check=True,
                )

            qmov = None
            for s in range(NSG):
                # one gather-stream DMA per SG: 4KB descriptors at full DMA
                # bus rate, and the PE only ever waits on one SG's transfer
                gt = gbuf.tile([128, 2, B], F8, tag="g", name=f"gt{s}")
                nc.sync.dma_start(gt[:], gath[:, s, :, :])
                if s == 0:
                    nc.sync.dma_start(qwt_sb[:, 0:4, :, :], qwt[:, 0:4, :, :])
                if s == 8:
                    nc.sync.dma_start(qwt_sb[:, 4:16, :, :], qwt[:, 4:16, :, :])
                if s == 2:
                    nc.sync.dma_start(wbd_sb[:, 2:8, :, :], wbd[:, 2:8, :, :])
                if s == 5:
                    nc.sync.dma_start(wbd_sb[:, 8:20, :, :], wbd[:, 8:20, :, :])
                if s == 11:
                    nc.sync.dma_start(wbd_sb[:, 20:32, :, :], wbd[:, 20:32, :, :])

                sp = s % 2
                last = s == NSG - 1
                if sp == 0:
                    qmov = fbuf.tile([128, 2, B], F8, tag="f", name=f"qmov{s // 2}")
                # stage 1: four [128, 512] PSUM tiles (2 chunks each);
                # 1-bank slots give the MM->evict->MM loop 4-deep rotation
                for pt in range(4):
                    pre = pre_ps.tile([128, 512], F32, tag="pre", name=f"pre{s}_{pt}")
                    for k in range(2):
                        c0 = 512 * pt + CH * k
                        nc.tensor.matmul(
                            pre[:, CH * k : CH * (k + 1)],
                            wbd_sb[:, s, :, :],
                            gt[:, :, c0 : c0 + CH],
                            start=(k == 0),
                            stop=(k == 1),
                            perf_mode=DR,
                            tile_position=(0, 0),
                            skip_group_check=True,
                        )
                    evict(qmov[:, sp, 512 * pt : 512 * (pt + 1)], pre[:])
                    if last:
                        # drain: interleave the final pair's Q matmuls with
                        # its evictions, emitting each output half early
                        q_mm(s // 2, qmov, 2 * pt)
                        q_mm(s // 2, qmov, 2 * pt + 1)
                        if pt == 1:
                            emit_qout(0)
                        if pt == 3:
                            emit_qout(1)
                if sp == 1 and not last:
                    q_flush(s // 2, qmov)

    nc.finalize()
    return nc


def _get_program():
    global _PROGRAM
    if _PROGRAM is None:
        _PROGRAM = _build_program()
    return _PROGRAM


def _stage_inputs(observation, gvf_W, q_W, gvf_input_idxs):
    """Host-side sharding/layout. Returns in_maps (list of dicts, one per core).

    Row mapping within an SG: r in [0,256) -> (g_local, i) = (r//16, r%16),
    stored at (sub, p) = (r//128, r%128).  Stationary col = 8*g_local + h.
    """
    obs = np.asarray(observation, dtype=np.float32)
    gw = np.asarray(gvf_W, dtype=np.float32)
    qw = np.asarray(q_W, dtype=np.float32)
    idx = np.asarray(gvf_input_idxs).astype(np.int64)

    obsT8 = np.ascontiguousarray(obs.T).astype(ml_dtypes.float8_e4m3)

    p = np.arange(128)
    sub = np.arange(2)
    r = 128 * sub[None, :] + p[:, None]  # (128, 2)
    gl_of_r = r // IPG  # gvf-in-SG
    i_of_r = r % IPG  # input slot

    in_maps = []
    for c in range(N_CORES):
        gv0 = c * GPC
        ss = np.arange(NSG)

        # gath[p, s, sub, b] = obs8[idx[gv0+16s+gl(r), i(r)], b]
        g_idx = gv0 + SGV * ss[None, :, None] + gl_of_r[:, None, :]  # (128,32,2)
        d_idx = idx[g_idx, i_of_r[:, None, :]]  # (128,32,2)
        gath_h = obsT8[d_idx]  # (128,32,2,B)

        # wbd[p, s, sub, 8gl+h] = SW * W[gv0+16s+gl, h, i] at (sub,p)<->(gl,i)
        wbd_h = np.zeros((128, NSG, 2, 128), dtype=np.float32)
        hh = np.arange(HPG)
        # scatter: for each (p, s, sub): gl = gl_of_r[p,sub], i = i_of_r[p,sub]
        pp_b = np.broadcast_to(p[:, None, None, None], (128, NSG, 2, HPG))
        ss_b = np.broadcast_to(ss[None, :, None, None], (128, NSG, 2, HPG))
        sub_b = np.broadcast_to(sub[None, None, :, None], (128, NSG, 2, HPG))
        gl_b = np.broadcast_to(gl_of_r[:, None, :, None], (128, NSG, 2, HPG))
        i_b = np.broadcast_to(i_of_r[:, None, :, None], (128, NSG, 2, HPG))
        hh_b = np.broadcast_to(hh[None, None, None, :], (128, NSG, 2, HPG))
        vals = SW * gw[gv0 + SGV * ss_b + gl_b, hh_b, i_b]
        wbd_h[pp_b, ss_b, sub_b, 8 * gl_b + hh_b] = vals
        wbd_h = wbd_h.astype(ml_dtypes.float8_e4m3)

        # qwt[p, t, sub, a] = SW * qw[a, OBS + 8*(gv0 + 16*(2t+sub) + p//8) + p%8]
        # (a >= NA zero-padded: dual-fp8 needs the full 128x128 PE tile)
        tt = np.arange(NPAIR)
        g_of = gv0 + SGV * (2 * tt[None, :, None] + sub[None, None, :]) + (
            p[:, None, None] // HPG
        )  # (128, 16, 2)
        colf = OBS_DIM + g_of * HPG + (p[:, None, None] % HPG)
        qwt_h = np.zeros((128, NPAIR, 2, 128), dtype=np.float32)
        qwt_h[:, :, :, :NA] = (SW * qw[:, colf]).transpose(1, 2, 3, 0)
        qwt_h = qwt_h.astype(ml_dtypes.float8_e4m3)

        in_maps.append(
            {
                "gath": np.ascontiguousarray(gath_h),
                "wbd": np.ascontiguousarray(wbd_h),
                "qwt": np.ascontiguousarray(qwt_h),
            }
        )
    return in_maps


def _emulate_core(m):
    """Numpy emulation of the device program for one core's in_map.
    Returns qp [128, B] f32 (strip layout, pre-1/SQ-scaled output)."""
    gath = m["gath"].astype(np.float32)  # (128,32,2,B)
    wbd = m["wbd"].astype(np.float32)  # (128,32,2,128)
    qwt = m["qwt"].astype(np.float32)  # (128,16,2,128)
    qacc = np.zeros((128, B), dtype=np.float32)
    for t in range(NPAIR):
        qmov = np.zeros((128, 2, B), dtype=np.float32)
        for spp in range(2):
            s = 2 * t + spp
            pre = np.einsum("pub,pum->mb", gath[:, s], wbd[:, s])
            feat8 = np.maximum(pre, 0.0).astype(ml_dtypes.float8_e4m3)
            qmov[:, spp, :] = feat8.astype(np.float32)
        qacc += np.einsum("pub,pua->ab", qmov, qwt[:, t])  # (128, B)
    return (qacc[0:NA] / SQ).astype(np.float32)


def kernel(observation, gvf_W, q_W, gvf_input_idxs, _trace=False, _emulate=False):
    in_maps = _stage_inputs(observation, gvf_W, q_W, gvf_input_idxs)
    if _emulate:
        results = [_emulate_core(m) for m in in_maps]
    else:
        nc = _get_program()
        res = run_bass_kernel_spmd(nc, in_maps, list(range(N_CORES)), trace=_trace)
        results = [res.results[c]["qp"].astype(np.float32) for c in range(N_CORES)]
        if _trace:
            kernel.last_exec_time_ns = res.exec_time_ns
    qacc = np.zeros((NA, B), dtype=np.float32)
    for qpc in results:
        qacc += qpc
    # obs part of the Q head (1% of FLOPs, exact f32) on host
    obs = np.asarray(observation, dtype=np.float32)
    qw = np.asarray(q_W, dtype=np.float32)
    out = qacc.T + obs @ qw[:, :OBS_DIM].T
    return np.ascontiguousarray(out, dtype=np.float32)
